# revision 12
# baseline (speedup 1.0000x reference)
"""Trainium2 Bass kernel for ConvLayer: 3x3 same-padding conv, N=32 C=192
H=W=56 Cout=384, fp32, + bias.

Strategy: data-parallel over batch across 8 NeuronCores (4 images/core, no
collectives). Per core the conv is an implicit GEMM on the TensorEngine.
The contraction folds (kh, ci) pairs into the partition dim: 3*192 = 576
values = 5 chunks of 128 (last chunk half zero-weighted), so each output
block needs 5 chunks x 3 kw taps = 15 accumulating matmuls instead of the
naive 9 taps x 2 channel chunks = 18. The kh row shift is baked into the
SBUF x layout at DMA time (chunk q, partition p holds rows shifted by the
pair's kh). Output channels (384) = 3 partition chunks of 128; matmul free
dim = 8 output rows x 56 cols = 448 pixels into one PSUM bank.

Inputs are spatially zero-padded to 58x58 on the host; weights are
pre-transposed on the host into the lhsT layout [pair_in_chunk, chunk, kw,
mc, cout_in_chunk]; both are typed float32r (fp32 bits, full-rate PE mode).
"""

import sys

sys.path.insert(0, "/opt/trn_rl_repo")

import numpy as np

import concourse.bass as bass  # noqa: F401
import concourse.tile as tile
from concourse import bacc, mybir
from concourse.bass_utils import run_bass_kernel_spmd
from concourse.tile_rust import add_dep_helper

N, C, H, W_ = 32, 192, 56, 56
COUT = 384
NCORES = 8
NPC = N // NCORES  # images per core
HP, WP = H + 2, W_ + 2  # 58x58 padded
NQ = 5  # (kh, ci) contraction chunks of 128 (5*128 = 640 >= 576)
MC = 3  # cout chunks of 128
RB = 8  # output rows per psum block
NBLK = H // RB  # 7
NPAIR = 3 * C  # 576 (kh-major: pair = kh*C + ci)

MM_DT = mybir.dt.float32r

# Affine source pieces for each contraction chunk q: list of
# (dst_part_lo, dst_part_hi, kh, ci_lo, ci_hi). Chunk q covers pairs
# [128q, 128q+128); pair = kh*C + ci.
_CHUNK_PIECES = []
for _q in range(NQ):
    pieces = []
    lo, hi = _q * 128, _q * 128 + 128
    p = lo
    while p < hi:
        if p >= NPAIR:
            # zero-weighted tail: map to kh=2 data (weights are 0 there)
            pieces.append((p - lo, hi - lo, 2, C - (hi - p), C))
            break
        kh, ci = p // C, p % C
        run = min(hi, (kh + 1) * C) - p
        pieces.append((p - lo, p - lo + run, kh, ci, ci + run))
        p += run
    _CHUNK_PIECES.append(pieces)

_NC_CACHE = {}


def _build():
    f32 = mybir.dt.float32
    nc = bacc.Bacc("TRN2", target_bir_lowering=False, debug=False)
    xin = nc.dram_tensor("x", [NPC, C, HP, WP], MM_DT, kind="ExternalInput").ap()
    win = nc.dram_tensor("w", [128, MC, NQ, 3, 128], MM_DT, kind="ExternalInput").ap()
    bin_ = nc.dram_tensor("b", [128, MC], f32, kind="ExternalInput").ap()
    out = nc.dram_tensor("out", [NPC, COUT, H, W_], f32, kind="ExternalOutput").ap()

    with tile.TileContext(nc) as tc:
        with (
            tc.tile_pool(name="wpool", bufs=1) as wpool,
            tc.tile_pool(name="xpool", bufs=2) as xpool,
            tc.tile_pool(name="opool", bufs=4) as opool,
            tc.tile_pool(name="ppool", bufs=8, space="PSUM") as ppool,
        ):
            # Input DMAs are chained (each waits on the previous) so they
            # complete in consumption order: the first matmul can start
            # after ~2MB instead of after the whole 12MB startup set, which
            # would otherwise race across all 8 DMA queues.
            prev_dma = [None]

            def chained_dma(dst, src):
                inst = nc.sync.dma_start(dst, src)
                if prev_dma[0] is not None:
                    add_dep_helper(
                        inst.ins, prev_dma[0], sync=True, reason="input dma chain"
                    )
                prev_dma[0] = inst.ins
                return inst

            # Per-mc weight tiles so mc=0 matmuls start after a 1.2MB DMA.
            w_m = []
            for mc in range(MC):
                wt = wpool.tile([128, NQ, 3, 128], MM_DT, name=f"w{mc}", tag=f"w{mc}")
                w_m.append(wt)
            b_t = wpool.tile([128, MC], f32)
            chained_dma(w_m[0][:], win[:, 0])
            chained_dma(b_t[:], bin_[:])

            def load_image(img):
                # Per-chunk x tiles: chunk q, partition p holds 56 rows of
                # xpad[ci] shifted down by the pair's kh, all 58 cols.
                xq = []
                for q in range(NQ):
                    xb = xpool.tile([128, H, WP], MM_DT, name=f"x{q}", tag=f"x{q}")
                    for (p0, p1, kh, c0, c1) in _CHUNK_PIECES[q]:
                        chained_dma(xb[p0:p1], xin[img, c0:c1, kh : kh + H, :])
                    xq.append(xb)
                return xq

            xq = load_image(0)
            chained_dma(w_m[1][:], win[:, 1])
            chained_dma(w_m[2][:], win[:, 2])

            for img in range(NPC):
                if img > 0:
                    xq = load_image(img)
                for mc in range(MC):
                    psums = [
                        ppool.tile([128, RB, W_], f32, name=f"ps{i}", tag="ps")
                        for i in range(NBLK)
                    ]
                    t = 0
                    nmm = NQ * 3
                    for q in range(NQ):
                        for kw in range(3):
                            lhsT = w_m[mc][:, q, kw, :]
                            for blk in range(NBLK):
                                rhs = xq[q][
                                    :, blk * RB : blk * RB + RB, kw : kw + W_
                                ]
                                nc.tensor.matmul(
                                    psums[blk][:],
                                    lhsT,
                                    rhs,
                                    start=(t == 0),
                                    stop=(t == nmm - 1),
                                )
                            t += 1
                    for blk in range(NBLK):
                        ot = opool.tile([128, RB, W_], f32)
                        if blk % 2 == 0:
                            nc.scalar.activation(
                                ot[:],
                                psums[blk][:],
                                mybir.ActivationFunctionType.Identity,
                                bias=b_t[:, mc : mc + 1],
                            )
                        else:
                            nc.vector.tensor_scalar_add(
                                ot[:], psums[blk][:], b_t[:, mc : mc + 1]
                            )
                        nc.sync.dma_start(
                            out[
                                img,
                                mc * 128 : (mc + 1) * 128,
                                blk * RB : (blk + 1) * RB,
                                :,
                            ],
                            ot[:],
                        )
    nc.compile()
    return nc


def _get_nc():
    if "nc" not in _NC_CACHE:
        _NC_CACHE["nc"] = _build()
    return _NC_CACHE["nc"]


def _prep_in_maps(x, W, b):
    x = np.asarray(x, dtype=np.float32)
    W = np.asarray(W, dtype=np.float32)
    b = np.asarray(b, dtype=np.float32)

    # Spatial zero-pad to 58x58.
    xp = np.zeros((N, C, HP, WP), np.float32)
    xp[:, :, 1 : H + 1, 1 : W_ + 1] = x

    # lhsT weights [pair_in_chunk, q, kw, mc, co]; pair = kh*C + ci.
    wtr = W.transpose(1, 2, 3, 0)  # [ci, kh, kw, co]
    wpairs = np.zeros((NQ * 128, 3, COUT), np.float32)  # [pair, kw, co]
    wpairs[:NPAIR] = wtr.transpose(1, 0, 2, 3).reshape(NPAIR, 3, COUT)
    wt = np.ascontiguousarray(
        wpairs.reshape(NQ, 128, 3, MC, 128).transpose(1, 3, 0, 2, 4)
    )

    bh = np.ascontiguousarray(b.reshape(MC, 128).T)  # [co_in_chunk, mc]

    return [
        {"x": xp[i * NPC : (i + 1) * NPC], "w": wt, "b": bh} for i in range(NCORES)
    ]


def kernel(x, W, b):
    nc = _get_nc()
    in_maps = _prep_in_maps(x, W, b)
    res = run_bass_kernel_spmd(nc, in_maps, core_ids=list(range(NCORES)))
    return np.concatenate(
        [res.results[i]["out"] for i in range(NCORES)], axis=0
    )


# revision 14
# speedup vs baseline: 1.0690x; 1.0690x over previous
"""Trainium2 Bass kernel for ConvLayer: 3x3 same-padding conv, N=32 C=192
H=W=56 Cout=384, fp32, + bias.

Strategy: data-parallel over batch across 8 NeuronCores (4 images/core, no
collectives). Per core the conv is an implicit GEMM on the TensorEngine.
The contraction folds (kh, ci) pairs into the partition dim: 3*192 = 576
values = 5 chunks of 128 (last chunk half zero-weighted), so each output
block needs 5 chunks x 3 kw taps = 15 accumulating matmuls instead of the
naive 9 taps x 2 channel chunks = 18. The kh row shift is baked into the
SBUF x layout at DMA time (chunk q, partition p holds rows shifted by the
pair's kh). Output channels (384) = 3 partition chunks of 128; matmul free
dim = 8 output rows x 56 cols = 448 pixels into one PSUM bank.

Inputs are spatially zero-padded to 58x58 on the host; weights are
pre-transposed on the host into the lhsT layout [pair_in_chunk, chunk, kw,
mc, cout_in_chunk]; both are typed float32r (fp32 bits, full-rate PE mode).
"""

import sys

sys.path.insert(0, "/opt/trn_rl_repo")

import numpy as np

import concourse.bass as bass  # noqa: F401
import concourse.tile as tile
from concourse import bacc, mybir
from concourse.bass_utils import run_bass_kernel_spmd
from concourse.tile_rust import add_dep_helper

N, C, H, W_ = 32, 192, 56, 56
COUT = 384
NCORES = 8
NPC = N // NCORES  # images per core
HP, WP = H + 2, W_ + 2  # 58x58 padded
NQ = 5  # (kh, ci) contraction chunks of 128 (5*128 = 640 >= 576)
MC = 3  # cout chunks of 128
RB = 8  # output rows per psum block
NBLK = H // RB  # 7
NPAIR = 3 * C  # 576 (kh-major: pair = kh*C + ci)

MM_DT = mybir.dt.float32r

# Affine source pieces for each contraction chunk q: list of
# (dst_part_lo, dst_part_hi, kh, ci_lo, ci_hi). Chunk q covers pairs
# [128q, 128q+128); pair = kh*C + ci.
_CHUNK_PIECES = []
for _q in range(NQ):
    pieces = []
    lo, hi = _q * 128, _q * 128 + 128
    p = lo
    while p < hi:
        if p >= NPAIR:
            # zero-weighted tail: map to kh=2 data (weights are 0 there)
            pieces.append((p - lo, hi - lo, 2, C - (hi - p), C))
            break
        kh, ci = p // C, p % C
        run = min(hi, (kh + 1) * C) - p
        pieces.append((p - lo, p - lo + run, kh, ci, ci + run))
        p += run
    _CHUNK_PIECES.append(pieces)

_NC_CACHE = {}


def _build():
    f32 = mybir.dt.float32
    nc = bacc.Bacc("TRN2", target_bir_lowering=False, debug=False)
    xin = nc.dram_tensor("x", [NPC, C, HP, WP], MM_DT, kind="ExternalInput").ap()
    win = nc.dram_tensor("w", [128, MC, NQ, 3, 128], MM_DT, kind="ExternalInput").ap()
    bin_ = nc.dram_tensor("b", [128, MC], f32, kind="ExternalInput").ap()
    out = nc.dram_tensor("out", [NPC, COUT, H, W_], f32, kind="ExternalOutput").ap()

    with tile.TileContext(nc) as tc:
        with (
            tc.tile_pool(name="wpool", bufs=1) as wpool,
            tc.tile_pool(name="xpool", bufs=2) as xpool,
            tc.tile_pool(name="opool", bufs=4) as opool,
            tc.tile_pool(name="ppool", bufs=8, space="PSUM") as ppool,
        ):
            # Input DMAs are sliced into ~400KB pieces and issued in
            # consumption order: the 8 HWDGE lanes are FIFO and round-robin
            # assigned, so small slices complete roughly in issue order at
            # full aggregate bandwidth. The first matmul can then start
            # after ~2MB instead of after the whole 12MB startup set.
            ROWSLC = 14  # rows per x DMA slice (128 part x 14 x 58 = 415KB)

            # Per-mc weight tiles so mc=0 matmuls start after a 1.2MB DMA.
            w_m = []
            for mc in range(MC):
                wt = wpool.tile([128, NQ, 3, 128], MM_DT, name=f"w{mc}", tag=f"w{mc}")
                w_m.append(wt)
            b_t = wpool.tile([128, MC], f32)

            def load_w(mc):
                for q in range(NQ):
                    nc.sync.dma_start(w_m[mc][:, q], win[:, mc, q])

            def load_image(img):
                # Per-chunk x tiles: chunk q, partition p holds 56 rows of
                # xpad[ci] shifted down by the pair's kh, all 58 cols.
                xq = []
                for q in range(NQ):
                    xb = xpool.tile([128, H, WP], MM_DT, name=f"x{q}", tag=f"x{q}")
                    for (p0, p1, kh, c0, c1) in _CHUNK_PIECES[q]:
                        rstep = ROWSLC if p1 - p0 > 64 else 2 * ROWSLC
                        for r0 in range(0, H, rstep):
                            r1 = min(r0 + rstep, H)
                            nc.sync.dma_start(
                                xb[p0:p1, r0:r1],
                                xin[img, c0:c1, kh + r0 : kh + r1, :],
                            )
                    xq.append(xb)
                return xq

            load_w(0)
            nc.sync.dma_start(b_t[:], bin_[:])
            xq = load_image(0)
            load_w(1)
            load_w(2)

            for img in range(NPC):
                if img > 0:
                    xq = load_image(img)
                for mc in range(MC):
                    psums = [
                        ppool.tile([128, RB, W_], f32, name=f"ps{i}", tag="ps")
                        for i in range(NBLK)
                    ]
                    t = 0
                    nmm = NQ * 3
                    for q in range(NQ):
                        for kw in range(3):
                            lhsT = w_m[mc][:, q, kw, :]
                            for blk in range(NBLK):
                                rhs = xq[q][
                                    :, blk * RB : blk * RB + RB, kw : kw + W_
                                ]
                                nc.tensor.matmul(
                                    psums[blk][:],
                                    lhsT,
                                    rhs,
                                    start=(t == 0),
                                    stop=(t == nmm - 1),
                                )
                            t += 1
                    for blk in range(NBLK):
                        ot = opool.tile([128, RB, W_], f32)
                        nc.scalar.activation(
                            ot[:],
                            psums[blk][:],
                            mybir.ActivationFunctionType.Identity,
                            bias=b_t[:, mc : mc + 1],
                        )
                        nc.sync.dma_start(
                            out[
                                img,
                                mc * 128 : (mc + 1) * 128,
                                blk * RB : (blk + 1) * RB,
                                :,
                            ],
                            ot[:],
                        )
    nc.compile()
    return nc


def _get_nc():
    if "nc" not in _NC_CACHE:
        _NC_CACHE["nc"] = _build()
    return _NC_CACHE["nc"]


def _prep_in_maps(x, W, b):
    x = np.asarray(x, dtype=np.float32)
    W = np.asarray(W, dtype=np.float32)
    b = np.asarray(b, dtype=np.float32)

    # Spatial zero-pad to 58x58.
    xp = np.zeros((N, C, HP, WP), np.float32)
    xp[:, :, 1 : H + 1, 1 : W_ + 1] = x

    # lhsT weights [pair_in_chunk, q, kw, mc, co]; pair = kh*C + ci.
    wtr = W.transpose(1, 2, 3, 0)  # [ci, kh, kw, co]
    wpairs = np.zeros((NQ * 128, 3, COUT), np.float32)  # [pair, kw, co]
    wpairs[:NPAIR] = wtr.transpose(1, 0, 2, 3).reshape(NPAIR, 3, COUT)
    wt = np.ascontiguousarray(
        wpairs.reshape(NQ, 128, 3, MC, 128).transpose(1, 3, 0, 2, 4)
    )

    bh = np.ascontiguousarray(b.reshape(MC, 128).T)  # [co_in_chunk, mc]

    return [
        {"x": xp[i * NPC : (i + 1) * NPC], "w": wt, "b": bh} for i in range(NCORES)
    ]


def kernel(x, W, b):
    nc = _get_nc()
    in_maps = _prep_in_maps(x, W, b)
    res = run_bass_kernel_spmd(nc, in_maps, core_ids=list(range(NCORES)))
    return np.concatenate(
        [res.results[i]["out"] for i in range(NCORES)], axis=0
    )
